# revision 1
# baseline (speedup 1.0000x reference)
"""Dilated MHSA block on 8 Trainium2 NeuronCores.

Sharding: sequence-parallel. Core c (0..7) handles batch b=c//4, query chunk
[512*(c%4), 512*(c%4)+512) with a 16-token halo of keys/values on each side.
Each core computes its full 512x1024 output slice; the host just concatenates.

All heavy matmuls take fp16 inputs (1 cycle/row on the PE at any moving dim;
an fp16 input carries the same 11-bit significand the PE's tf32/fp32r mode
would keep from fp32) and accumulate in fp32 PSUM, so inputs ship as fp16 -
half the DMA traffic - at fp32r-equivalent accuracy (~5e-4 rel err).
Normalization scalars stay fp32/fp32r.

Per-core device pipeline:
  1a. q,k projection qkT = Wqk @ x^T in (feature, token) layout, one weight
      DMA per feature-tile pair; ACT evicts PSUM with per-feature bias
      (Identity) and squares (Square); per-pair norm reductions via selector
      matmuls are software-pipelined one tile behind the main matmuls.
  1b. QK-norm: ACT sqrt, DVE eps+reciprocal; q is normalized via a rank-2
      broadcast matmul + DVE multiply; k's factor is folded into the exp
      scale (per-partition in keys-on-partitions layout) after PE-transposing
      the (16, 544) factor block to (token, head) tiles.
  1c. v projection in natural (token, feat) layout with token tiles shifted
      -16 so banded attention reads aligned v tiles; heads stored at stride
      65 with a ones column so the AV matmul also emits the softmax
      denominator. Overlapped with the j=0 attention sweep (PE-heavy vs
      DVE/ACT-heavy) under a shared PSUM budget.
  2.  Per (head, 256-query block): 3 score matmuls restricted to each key
      chunk's live query window (keys 128/128/32 on partitions, 160/160/32
      q free), one merged DVE mask-add over the two live 160-wide windows
      (strided 3D AP) + a (32, 32) add for the tail chunk, ACT exp on live
      windows only (scale = 1/(|k|+eps)), 3 AV matmuls; adjacent head pairs
      pack their AV outputs into one (65, 512) PSUM bank so the DVE
      reciprocal, rank-1 broadcast matmul, and ACT evict run once per pair
      at 512 width, then per-head DVE multiplies -> normalized outT.
      P-tile dead regions are zeroed once via DMA and never rewritten.
  3.  Output projection y = outT^T @ out_w^T + b (bias as rank-1 K-append),
      interleaved per query block with the attention sweeps so the PE-bound
      projection hides under elementwise-bound attention.
"""

import os
import sys

for _p in ("/opt/trn_rl_repo", "/root/.axon_site/_ro/trn_rl_repo"):
    if os.path.isdir(_p) and _p not in sys.path:
        sys.path.insert(0, _p)

import numpy as np

import concourse.bass as bass
import concourse.mybir as mybir
import concourse.tile as tile
from concourse import bacc
from concourse import bass_utils

F32 = mybir.dt.float32
FR = mybir.dt.float32r  # fp32 bits, tf32 matmul mode (full rate at N >= 256)
F16 = mybir.dt.float16

B, N, D = 2, 2048, 1024
H, DH = 16, 64
KWIN, DIL = 8, 2
EPS = 1e-6
NCORES = 8
CHUNK = 512          # queries per core
HALO = 16            # KWIN * DIL
LOCAL = CHUNK + 2 * HALO  # 544 tokens (keys/values) per core
NEG = -30000.0       # mask value (exp(NEG * scale) == 0 for any sane scale)
KT = D // 128        # 8 contraction tiles


def _win(ap2d, stride, count, width):
    """(P, count, width) windowed view over a 2D AP's free dim."""
    return bass.AP(
        tensor=ap2d.tensor,
        offset=ap2d.offset,
        ap=[list(ap2d.ap[0]), [stride, count], [1, width]],
    )


def _emit(tc, T):
    nc = tc.nc
    AF = mybir.ActivationFunctionType
    OP = mybir.AluOpType

    with tc.tile_pool(name="persist", bufs=1) as pp:
        # ---- early loads -------------------------------------------------
        xT = pp.tile([128, KT, LOCAL], F16)          # x^T, (in-feat, token)
        for kt in range(KT):
            nc.sync.dma_start(xT[:, kt], T["xT"][128 * kt : 128 * (kt + 1)])

        wq_prefetch = {}

        qb2 = pp.tile([128, 16], F32)                # qkv bias for q,k
        sel = pp.tile([128, 2], F16)                  # head-pair sum selector
        sel16 = pp.tile([128, 8, 16], F16)            # k-side scatter selectors
        selT = pp.tile([2, 128], FR)                 # head-pair bcast selector
        ones1 = pp.tile([1, 128], F16)
        ones1r = pp.tile([1, 64], FR)
        ident = pp.tile([16, 16], F32)

        v_sb = pp.tile([128, 5, H * 65], F16)         # v natural, 65-stride heads
        v_h = v_sb.rearrange("p m (h c) -> p m h c", c=65)

        pabs = [pp.tile([128, 512], F16, name=f"pab{i}") for i in range(6)]
        pcs = [pp.tile([32, 256], F16, name=f"pc{i}") for i in range(6)]
        masks = pp.tile([128, 2, 3, 256], F16)       # (p, qblock, chunk, q)
        vb = pp.tile([1, D], F16)
        ob = pp.tile([1, D], F16)

        k_sb = pp.tile([128, 8, LOCAL], F16)          # k^T (biased), 2 heads/tile
        qn_sb = pp.tile([128, 8, LOCAL], F16)         # q^T normalized
        outTn = pp.tile([128, KT, CHUNK], F16)        # attn out^T (feat, q)
        rkT = pp.tile([128, 5, H], F32)              # 1/(|k|+eps), (token, head)
        rk = pp.tile([16, LOCAL], F32)

        # ---- phase 1a/1b: q,k projection + QK norm ----------------------
        with (
            tc.tile_pool(name="wpool", bufs=8) as wpool,
            tc.tile_pool(name="sqpool", bufs=6) as sqpool,
            tc.tile_pool(name="ps1", bufs=3, space="PSUM") as ps1,
            tc.tile_pool(name="bcp", bufs=1, space="PSUM") as bcp,
        ):
            for mp in range(2):
                wt = wpool.tile([128, KT, 256], F16, tag="wqk", name=f"wtp{mp}")
                nc.sync.dma_start(
                    wt,
                    T["wqk"][:, 256 * mp : 256 * (mp + 1)].rearrange(
                        "(kt p) n -> p kt n", p=128
                    ),
                )
                wq_prefetch[mp] = wt
            nc.sync.dma_start(qb2, T["qb2"].rearrange("(m p) -> p m", p=128))
            nc.sync.dma_start(sel, T["sel"])
            nc.sync.dma_start(sel16, T["sel16"].rearrange("g p c -> p g c"))
            nc.sync.dma_start(selT, T["selT"])
            nc.sync.dma_start(ones1, T["ones1"])
            nc.sync.dma_start(ones1r, T["ones1r"])
            nsk = None
            pending = []

            def epilogue(m, sqv):
                nonlocal nsk
                g = m % 8
                if m < 8:
                    nps = bcp.tile([2, 1024], F32, tag="bcnsk", name="nsq")
                    for hf in range(2):
                        nc.tensor.matmul(
                            nps[:, 512 * hf : 512 * hf + 256],
                            sel,
                            sqv[:, hf],
                            start=True,
                            stop=True,
                        )
                    npsv = nps.rearrange("p (b c) -> p b c", c=512)[:, :, 0:256]
                    rpair = sqpool.tile([2, 2, 256], FR, tag="rpair")
                    nc.scalar.activation(rpair, npsv, AF.Sqrt)
                    nc.vector.tensor_scalar_add(rpair, rpair, EPS)
                    nc.vector.reciprocal(rpair, rpair)
                    # normalize q: qn *= bcast(rq) over the pair's 128 rows
                    bc = bcp.tile([128, 1024], F32, tag="bcnsk", name="bc")
                    dstv = qn_sb[:, g][:, 16:528].rearrange(
                        "p (b c) -> p b c", c=256
                    )
                    for hf in range(2):
                        nc.tensor.matmul(
                            bc[:, 512 * hf : 512 * hf + 256],
                            selT,
                            rpair[:, hf],
                            start=True,
                            stop=True,
                        )
                    bcv = bc.rearrange("p (b c) -> p b c", c=512)[:, :, 0:256]
                    nc.vector.tensor_tensor(dstv, dstv, bcv, OP.mult)
                else:
                    # scatter-accumulate all 16 k sumsq rows into one tile
                    if nsk is None:
                        nsk = bcp.tile([16, 1024], F32, tag="bcnsk", name="nsk")
                    for hf in range(2):
                        nc.tensor.matmul(
                            nsk[:, 512 * hf : 512 * hf + 272],
                            sel16[:, g],
                            sqv[:, hf],
                            start=(g == 0),
                            stop=(g == 7),
                        )

            for mp in range(8):          # pair of feature tiles
                is_q = mp < 4
                # q features need only the 512 live query columns [16:528);
                # k features need all 544 (keys include the halo)
                w = 256 if is_q else 272
                x0 = 16 if is_q else 0
                pss = [
                    ps1.tile([128, 1024], F32, tag="qkps", name=f"qkps{i}")
                    for i in range(2)
                ]
                if mp in wq_prefetch:
                    wt = wq_prefetch.pop(mp)
                else:
                    wt = wpool.tile([128, KT, 256], F16, tag="wqk")
                    nc.sync.dma_start(
                        wt,
                        T["wqk"][:, 256 * mp : 256 * (mp + 1)].rearrange(
                            "(kt p) n -> p kt n", p=128
                        ),
                    )
                for kt in range(KT):
                    for mi in range(2):
                        for hf in range(2):
                            nc.tensor.matmul(
                                pss[mi][:, 512 * hf : 512 * hf + w],
                                wt[:, kt, 128 * mi : 128 * (mi + 1)],
                                xT[:, kt, x0 + w * hf : x0 + w * (hf + 1)],
                                start=(kt == 0),
                                stop=(kt == KT - 1),
                            )
                for mi in range(2):
                    m = 2 * mp + mi
                    g = m % 8
                    psv = pss[mi].rearrange("p (b c) -> p b c", c=512)[:, :, 0:w]
                    bias_col = qb2[:, m : m + 1]
                    dst = (qn_sb if m < 8 else k_sb)[:, g]
                    dstv = dst[:, x0 : x0 + 2 * w].rearrange(
                        "p (b c) -> p b c", c=w
                    )
                    nc.scalar.activation(dstv, psv, AF.Identity, bias=bias_col)
                    sq = sqpool.tile([128, LOCAL], F16, tag="sq")
                    nc.scalar.activation(
                        sq[:, 0 : 2 * w].rearrange("p (b c) -> p b c", c=w),
                        psv,
                        bias=bias_col,
                        func=AF.Square,
                    )
                    pending.append(
                        (m, sq[:, 0 : 2 * w].rearrange("p (b c) -> p b c", c=w))
                    )
                while len(pending) > 2:
                    epilogue(*pending.pop(0))
            while pending:
                epilogue(*pending.pop(0))
            nskv = nsk.rearrange("p (b c) -> p b c", c=512)[:, :, 0:272]
            nc.scalar.activation(
                rk.rearrange("p (b c) -> p b c", c=272), nskv, AF.Sqrt
            )
            nc.vector.tensor_scalar_add(rk, rk, EPS)
            nc.vector.reciprocal(rk, rk)

        # transpose rk (16, 544) -> rkT (token, head) tiles
        nc.sync.dma_start(ident, T["ident"])
        with tc.tile_pool(name="tp", bufs=2, space="PSUM") as tpp:
            for c in range(5):
                w = 128 if c < 4 else LOCAL - 512
                tp = tpp.tile([128, 16], F32, tag="tp")
                nc.tensor.transpose(tp[0:w], rk[:, 128 * c : 128 * c + w], ident)
                nc.vector.tensor_copy(rkT[0:w, c], tp[0:w])

        # ---- phase 1c + 2 + 3: v-proj overlapped with attention; ------
        # ---- out-projection interleaved per query block ----------------
        wv = pp.tile([128, KT, D], F16)              # Wv^T (in-feat, v-feat)
        nc.sync.dma_start(wv, T["wv"].rearrange("(kt p) n -> p kt n", p=128))
        nc.sync.dma_start(vb, T["vb"])
        nc.sync.dma_start(v_h[:, :, :, 64:65], T["vones"][:, :, :, None])
        nc.sync.dma_start(masks, T["masks"])
        for i in range(6):
            nc.sync.dma_start(pabs[i][:, 160:352], T["zeros"][:, 0:192])
            nc.sync.dma_start(pcs[i][:, 0:224], T["zeros"][0:32, 0:224])
        nc.sync.dma_start(ob, T["ob"])
        ow = pp.tile([128, KT, D], F16)              # out_w^T (feat, out)
        nc.sync.dma_start(ow, T["ow"].rearrange("(kt p) n -> p kt n", p=128))

        def v_tile(vps, m):
            M = 128 if m < 4 else LOCAL - 512
            vp = vps.tile([128, 1024], F32, tag="vps", name="vp")
            for nh in range(2):
                for kt in range(KT):
                    nc.tensor.matmul(
                        vp[0:M, 512 * nh : 512 * (nh + 1)],
                        xT[:, kt, 128 * m : 128 * m + M],
                        wv[:, kt, 512 * nh : 512 * (nh + 1)],
                        start=(kt == 0),
                        stop=False,
                    )
                nc.tensor.matmul(
                    vp[0:M, 512 * nh : 512 * (nh + 1)],
                    ones1[:, 0:M],
                    vb[:, 512 * nh : 512 * (nh + 1)],
                    start=False,
                    stop=True,
                )
                vpv = vp[0:M, 512 * nh : 512 * (nh + 1)].rearrange(
                    "p (h c) -> p h c", c=64
                )
                nc.vector.tensor_copy(v_h[0:M, m, 8 * nh : 8 * (nh + 1), 0:64], vpv)

        it = 0
        ot2_box = [None]

        def att(stp, scp, otp, dnp, rrp, h, j):
            nonlocal it
            g, a = h // 2, h % 2
            q0 = 256 * j
            kx = k_sb[64 * a : 64 * a + 64]
            qx = qn_sb[64 * a : 64 * a + 64]
            st = stp.tile([128, 512], F32, tag="st", name="st")
            sc = scp.tile([32, 256], F32, tag="sc", name="sc")
            # score matmuls compute only each chunk's live query window:
            # A keys see q [0:160), B keys q [96:256), C keys q [224:256)
            nc.tensor.matmul(
                st[:, 0:160],
                kx[:, g, q0 : q0 + 128],
                qx[:, g, 16 + q0 : 16 + q0 + 160],
                start=True, stop=True,
            )
            nc.tensor.matmul(
                st[:, 352:512],
                kx[:, g, q0 + 128 : q0 + 256],
                qx[:, g, 16 + q0 + 96 : 16 + q0 + 256],
                start=True, stop=True,
            )
            nc.tensor.matmul(
                sc[0:32, 224:256],
                kx[:, g, q0 + 256 : q0 + 288],
                qx[:, g, 16 + q0 + 224 : 16 + q0 + 256],
                start=True, stop=True,
            )
            pab = pabs[it % 6]
            pc = pcs[it % 6]
            it += 1
            mf = masks[:, j].rearrange("p c q -> p (c q)")
            nc.vector.tensor_tensor(
                _win(pab, 352, 2, 160),
                _win(st, 352, 2, 160),
                _win(mf, 352, 2, 160),
                OP.add,
            )
            nc.vector.tensor_tensor(
                pc[0:32, 224:256],
                sc[0:32, 224:256],
                masks[0:32, j, 2, 224:256],
                OP.add,
            )
            nc.scalar.activation(
                pab[:, 0:160], pab[:, 0:160], AF.Exp,
                scale=rkT[:, 2 * j, h : h + 1],
            )
            nc.scalar.activation(
                pab[:, 352:512], pab[:, 352:512], AF.Exp,
                scale=rkT[:, 2 * j + 1, h : h + 1],
            )
            nc.scalar.activation(
                pc[0:32, 224:256], pc[0:32, 224:256], AF.Exp,
                scale=rkT[0:32, 2 * j + 2, h : h + 1],
            )
            # AV for heads (2g, 2g+1) packs into one (65, 512) PSUM tile;
            # the normalization chain runs once per pair at 512 width.
            if a == 0:
                ot2_box[0] = otp.tile([65, 512], F32, tag="ot", name="ot")
            ot = ot2_box[0][:, 256 * a : 256 * (a + 1)]
            # A streams all 256 q (seeding has_written); B and C accumulate
            # only their live query windows.
            nc.tensor.matmul(
                ot, v_h[:, 2 * j, h], pab[:, 0:256], start=True, stop=False
            )
            nc.tensor.matmul(
                ot[:, 96:256], v_h[:, 2 * j + 1, h], pab[:, 352:512],
                start=False, stop=False,
            )
            nc.tensor.matmul(
                ot[:, 224:256], v_h[0:32, 2 * j + 2, h], pc[:, 224:256],
                start=False, stop=True,
            )
            if a == 1:
                ot2 = ot2_box[0]
                rr = rrp.tile([1, 512], FR, tag="rr", name="rr")
                nc.vector.reciprocal(rr, ot2[64:65])
                dn = dnp.tile([64, 512], F32, tag="dn", name="dn")
                nc.tensor.matmul(dn, ones1r, rr, start=True, stop=True)
                otS = rrp.tile([64, 512], F32, tag="otS", name="otS")
                nc.scalar.activation(otS, ot2[0:64], AF.Copy)
                for aa in range(2):
                    nc.vector.tensor_tensor(
                        outTn[64 * aa : 64 * aa + 64, g, q0 : q0 + 256],
                        otS[:, 256 * aa : 256 * (aa + 1)],
                        dn[:, 256 * aa : 256 * (aa + 1)],
                        OP.mult,
                    )

        def outproj(yps, ysbp, qb):
            yp = yps.tile([128, 1024], F32, tag="yps", name="yp")
            for nh in range(2):
                for kt in range(KT):
                    nc.tensor.matmul(
                        yp[:, 512 * nh : 512 * (nh + 1)],
                        outTn[:, kt, 128 * qb : 128 * (qb + 1)],
                        ow[:, kt, 512 * nh : 512 * (nh + 1)],
                        start=(kt == 0),
                        stop=False,
                    )
                nc.tensor.matmul(
                    yp[:, 512 * nh : 512 * (nh + 1)],
                    ones1,
                    ob[:, 512 * nh : 512 * (nh + 1)],
                    start=False,
                    stop=True,
                )
            ysb = ysbp.tile([128, 1024], F32, tag="ysb", name="ysb")
            nc.scalar.activation(ysb, yp, AF.Copy)
            nc.sync.dma_start(T["y"][128 * qb : 128 * (qb + 1)], ysb)

        with (
            tc.tile_pool(name="rr", bufs=6) as rrp,
            tc.tile_pool(name="ysb", bufs=3) as ysbp,
            tc.tile_pool(name="stp", bufs=2, space="PSUM") as stp,
            tc.tile_pool(name="scp", bufs=1, space="PSUM") as scp,
            tc.tile_pool(name="otp", bufs=2, space="PSUM") as otp,
            tc.tile_pool(name="dnp", bufs=1, space="PSUM") as dnp,
        ):
            with tc.tile_pool(name="vps", bufs=1, space="PSUM") as vps:
                for m in range(3):
                    v_tile(vps, m)
                for h in range(H):
                    if h == 0:
                        v_tile(vps, 3)
                    if h == 2:
                        v_tile(vps, 4)
                    att(stp, scp, otp, dnp, rrp, h, 0)
            with tc.tile_pool(name="yps", bufs=1, space="PSUM") as yps:
                outproj(yps, ysbp, 0)
                outproj(yps, ysbp, 1)
                for h in range(H):
                    att(stp, scp, otp, dnp, rrp, h, 1)
                outproj(yps, ysbp, 2)
                outproj(yps, ysbp, 3)


_PROGRAM = None


def _build_program():
    global _PROGRAM
    if _PROGRAM is not None:
        return _PROGRAM
    nc = bacc.Bacc(
        "TRN2",
        target_bir_lowering=False,
        debug=False,
        enable_asserts=False,
        num_devices=NCORES,
    )
    T = {}

    def inp(name, shape, dt=FR):
        T[name] = nc.dram_tensor(name, shape, dt, kind="ExternalInput").ap()

    inp("xT", (D, LOCAL), F16)
    inp("wqk", (D, 2 * D), F16)
    inp("wv", (D, D), F16)
    inp("ow", (D, D), F16)
    inp("masks", (128, 2, 3, 256), F16)
    inp("qb2", (2 * D,), F32)
    inp("vb", (1, D), F16)
    inp("ob", (1, D), F16)
    inp("sel", (128, 2), F16)
    inp("sel16", (8, 128, 16), F16)
    inp("selT", (2, 128))
    inp("ones1", (1, 128), F16)
    inp("ones1r", (1, 64))
    inp("vones", (128, 5, 16), F16)
    inp("zeros", (128, 224), F16)
    inp("ident", (16, 16), F32)
    T["y"] = nc.dram_tensor("y", (CHUNK, D), F32, kind="ExternalOutput").ap()

    with tile.TileContext(nc) as tc:
        with nc.allow_low_precision(reason="fp16/fp32r matmul pipeline"):
            _emit(tc, T)
    nc.compile()
    _PROGRAM = nc
    return nc


def _host_masks(c0):
    """masks[p, j, chunk, qq] for the core at chunk start c0."""
    out = np.full((2, 3, 128, 256), NEG, dtype=np.float32)
    for j in range(2):
        qtok = c0 + 256 * j + np.arange(256)[None, :]          # (1, 256)
        for ci, (base, rows) in enumerate(((0, 128), (128, 128), (256, 32))):
            ktok = c0 - HALO + 256 * j + base + np.arange(rows)[:, None]  # (rows, 1)
            diff = ktok - qtok
            ok = (
                (np.abs(diff) <= KWIN * DIL)
                & (diff % DIL == 0)
                & (ktok >= 0)
                & (ktok < N)
            )
            out[j, ci, :rows][ok] = 0.0
    return np.ascontiguousarray(out.transpose(2, 0, 1, 3)).astype(np.float16)


def _host_inputs(x, qkv_w, qkv_b, out_w, out_b):
    wqk = np.ascontiguousarray(qkv_w[: 2 * D].T.astype(np.float16))    # (D, 2D)
    wv = np.ascontiguousarray(qkv_w[2 * D :].T.astype(np.float16))     # (D, D)
    ow = np.ascontiguousarray(out_w.T.astype(np.float16))              # (D, D)
    qb2 = np.ascontiguousarray(qkv_b[: 2 * D])
    vb = np.ascontiguousarray(qkv_b[2 * D :].reshape(1, D).astype(np.float16))
    ob = np.ascontiguousarray(out_b.reshape(1, D).astype(np.float16))
    sel = np.zeros((128, 2), dtype=np.float16)
    sel[:64, 0] = 1.0
    sel[64:, 1] = 1.0
    selT = np.ascontiguousarray(sel.T.astype(np.float32))
    sel16 = np.zeros((8, 128, 16), dtype=np.float16)
    for g in range(8):
        sel16[g, :64, 2 * g] = 1.0
        sel16[g, 64:, 2 * g + 1] = 1.0
    ones1 = np.ones((1, 128), dtype=np.float16)
    ones1r = np.ones((1, 64), dtype=np.float32)
    vones = np.ones((128, 5, 16), dtype=np.float16)
    zeros = np.zeros((128, 224), dtype=np.float16)
    ident = np.eye(16, dtype=np.float32)

    in_maps = []
    for c in range(NCORES):
        b, i = divmod(c, 4)
        c0 = CHUNK * i
        xT = np.zeros((D, LOCAL), dtype=np.float16)
        lo, hi = max(0, c0 - HALO), min(N, c0 + CHUNK + HALO)
        xT[:, lo - (c0 - HALO) : hi - (c0 - HALO)] = x[b, lo:hi].T.astype(
            np.float16
        )
        in_maps.append(
            {
                "xT": xT,
                "wqk": wqk,
                "wv": wv,
                "ow": ow,
                "masks": _host_masks(c0),
                "qb2": qb2,
                "vb": vb,
                "ob": ob,
                "sel": sel,
                "sel16": sel16,
                "selT": selT,
                "ones1": ones1,
                "ones1r": ones1r,
                "vones": vones,
                "zeros": zeros,
                "ident": ident,
            }
        )
    return in_maps


def kernel(x, qkv_w, qkv_b, out_w, out_b):
    x = np.asarray(x, dtype=np.float32)
    qkv_w = np.asarray(qkv_w, dtype=np.float32)
    qkv_b = np.asarray(qkv_b, dtype=np.float32)
    out_w = np.asarray(out_w, dtype=np.float32)
    out_b = np.asarray(out_b, dtype=np.float32)

    nc = _build_program()
    in_maps = _host_inputs(x, qkv_w, qkv_b, out_w, out_b)
    res = bass_utils.run_bass_kernel_spmd(nc, in_maps, core_ids=list(range(NCORES)))

    out = np.empty((B, N, D), dtype=np.float32)
    for c in range(NCORES):
        b, i = divmod(c, 4)
        out[b, CHUNK * i : CHUNK * (i + 1)] = res.results[c]["y"]
    return out



# revision 42
# speedup vs baseline: 1.7225x; 1.7225x over previous
"""Dilated MHSA block on 8 Trainium2 NeuronCores.

Sharding: sequence-parallel. Core c (0..7) handles batch b=c//4, query chunk
[512*(c%4), 512*(c%4)+512) with a 16-token halo of keys/values on each side.
Each core computes its full 512x1024 output slice; the host just concatenates.

All heavy matmuls take fp16 inputs (1 cycle/row) and accumulate in fp32 PSUM.

Per-core device pipeline (v2 — engine-balanced rewrite):
  1a. q,k projection qkT = Wqk @ x^T in (feature, token) layout. ACT evicts
      PSUM with per-feature bias (Identity); the Pool engine squares the
      evicted fp16 values; one-hot selector matmuls scatter per-head sumsq
      rows into two PSUM accumulators (nsq16 for q heads, nsk16 for k heads).
      After the sweep: one sqrt+eps+recip chain per side (vs. per-pair),
      PE transposes rk -> rkT (token, head) for the exp scale trick, and
      per-pair broadcast matmuls + DVE multiplies normalize q.
  1b. v projection in natural (token, feat) layout, bias via rank-1 matmul,
      Pool-engine eviction into 65-stride heads with a ones column so the AV
      matmul also emits the softmax denominator.
  2.  Per (head, 256-query block): 3 score matmuls (keys 128/128/32 on
      partitions; the 32-key tail chunk lives at st[0:32, 320:352] of the
      same PSUM bank), 3 ACT exps (scale = 1/(|k|+eps) per key partition),
      3 DVE multiplies by a 0/1 fp16 mask (all-SBUF fp16 -> fast DVE mode),
      3 AV matmuls packing head pairs into one (65,512) PSUM bank. Pair
      epilogue: DVE reciprocal of the denominator row, rank-1 broadcast
      matmul, and DVE multiplies reading both PSUM operands directly.
      pab dead region [160:256) is zeroed once by memset and never rewritten.
  3.  Output projection y = outT^T @ out_w^T + b (bias as rank-1 matmul),
      ACT-evicted to fp16 and DMA'd out (host upcasts); interleaved per query
      block with the attention sweeps.
"""

import os
import sys

for _p in ("/opt/trn_rl_repo", "/root/.axon_site/_ro/trn_rl_repo"):
    if os.path.isdir(_p) and _p not in sys.path:
        sys.path.insert(0, _p)

import numpy as np

import concourse.bass as bass
import concourse.mybir as mybir
import concourse.tile as tile
from concourse import bacc
from concourse import bass_utils

F32 = mybir.dt.float32
FR = mybir.dt.float32r  # fp32 bits, tf32 matmul mode (full rate at N >= 256)
F16 = mybir.dt.float16

B, N, D = 2, 2048, 1024
H, DH = 16, 64
KWIN, DIL = 8, 2
EPS = 1e-6
NCORES = 8
CHUNK = 512          # queries per core
HALO = 16            # KWIN * DIL
LOCAL = CHUNK + 2 * HALO  # 544 tokens (keys/values) per core
KT = D // 128        # 8 contraction tiles


def _declare_io(nc):
    T = {}

    def inp(name, shape, dt=FR):
        T[name] = nc.dram_tensor(name, shape, dt, kind="ExternalInput").ap()

    inp("xT", (D, LOCAL), F16)
    inp("wqk", (D, 2 * D), F16)
    inp("wv", (D, D), F16)
    inp("ow", (D, D), F16)
    inp("masks", (128, 2, 512), F16)
    inp("qb2", (2 * D,), F32)
    inp("vbb", (128, D), F16)
    inp("ob", (1, D), F16)
    inp("sel16", (8, 128, 16), F16)
    inp("selT16", (16, 8, 128))
    inp("ones1", (1, 128), F16)
    inp("ones1r", (1, 64))
    inp("vones", (128, 5, 16), F16)
    inp("ident", (16, 16), F32)
    T["y"] = nc.dram_tensor("y", (CHUNK, D), F16, kind="ExternalOutput").ap()
    return T


def _emit(tc, T):
    nc = tc.nc
    AF = mybir.ActivationFunctionType
    OP = mybir.AluOpType

    with tc.tile_pool(name="persist", bufs=1) as pp:
        xT = pp.tile([128, KT, LOCAL], F16)          # x^T, (in-feat, token)
        wq_prefetch = {}

        qb2 = pp.tile([128, 16], F32)                # qkv bias for q,k
        sel16 = pp.tile([128, 8, 16], F16)            # per-head scatter selectors
        selT16 = pp.tile([16, 8, 128], FR)           # head-pair bcast selectors
        ones1 = pp.tile([1, 128], F16)
        ones1r = pp.tile([1, 64], FR)
        ident = pp.tile([16, 16], F32)

        v_sb = pp.tile([128, 5, H * 65], F16)         # v natural, 65-stride heads
        v_h = v_sb.rearrange("p m (h c) -> p m h c", c=65)

        pabs = [pp.tile([128, 512], F16, name=f"pab{i}") for i in range(6)]
        masks = pp.tile([128, 2, 512], F16)          # (p, qblock, col) 0/1
        vbb = pp.tile([128, D], F16)                 # v bias, host-broadcast
        ob = pp.tile([1, D], F16)

        k_sb = pp.tile([128, 8, LOCAL], F16)          # k^T (biased), 2 heads/tile
        qn_sb = pp.tile([128, 8, LOCAL], F16)         # q^T normalized
        outTn = pp.tile([128, KT, CHUNK], F16)        # attn out^T (feat, q)
        rkT = pp.tile([128, 5, H], F32)              # 1/(|k|+eps), (token, head)
        rk = pp.tile([16, LOCAL], F32)
        rq16 = pp.tile([16, CHUNK], FR)              # 1/(|q|+eps), (head, q)

        # zero the pab dead region [144:256) once; never rewritten
        for i in range(6):
            nc.vector.memset(pabs[i][:, 144:256], 0.0)

        # ---- phase 1a: q,k projection + norm reductions -----------------
        with (
            tc.tile_pool(name="wpool", bufs=8) as wpool,
            tc.tile_pool(name="sqpool", bufs=4) as sqpool,
            tc.tile_pool(name="ps1", bufs=2, space="PSUM") as ps1,
            tc.tile_pool(name="nsp", bufs=1, space="PSUM") as nsp,
        ):
            # startup-critical DMA order: first weight k-tile, then x k-tiles
            # interleaved with the rest of the first two weight tiles, so the
            # PE can start at ~1us instead of waiting for full transfers.
            wts = [
                wpool.tile([128, KT, 256], F16, tag="wqk", name=f"wtp{mp}")
                for mp in range(2)
            ]
            wq_prefetch.update(enumerate(wts))
            wqkv = T["wqk"].rearrange("(kt p) n -> p kt n", p=128)
            xTv = T["xT"].rearrange("(kt p) n -> p kt n", p=128)
            for k0, k1 in ((0, 2), (2, 4), (4, 8)):
                nc.sync.dma_start(wts[0][:, k0:k1], wqkv[:, k0:k1, 0:256])
                nc.sync.dma_start(xT[:, k0:k1], xTv[:, k0:k1])
                nc.sync.dma_start(wts[1][:, k0:k1], wqkv[:, k0:k1, 256:512])
            nc.sync.dma_start(qb2, T["qb2"].rearrange("(m p) -> p m", p=128))
            nc.sync.dma_start(sel16, T["sel16"].rearrange("g p c -> p g c"))
            nc.sync.dma_start(selT16, T["selT16"])
            nc.sync.dma_start(ones1, T["ones1"])
            nc.sync.dma_start(ones1r, T["ones1r"])
            nc.sync.dma_start(ident, T["ident"])

            nsq16 = nsp.tile([16, 1024], F32, name="nsq16")  # q sumsq (head, q)
            nsk16 = nsp.tile([16, 1024], F32, name="nsk16")  # k sumsq (head, tok)

            for mp in range(8):          # pair of feature tiles
                is_q = mp < 4
                # q features need only the 512 live query columns [16:528);
                # k features need all 544 (keys include the halo)
                w = 256 if is_q else 272
                x0 = 16 if is_q else 0
                pss = [
                    ps1.tile([128, 1024], F32, tag="qkps", name=f"qkps{i}")
                    for i in range(2)
                ]
                if mp in wq_prefetch:
                    wt = wq_prefetch.pop(mp)
                else:
                    wt = wpool.tile([128, KT, 256], F16, tag="wqk")
                    nc.sync.dma_start(
                        wt,
                        T["wqk"][:, 256 * mp : 256 * (mp + 1)].rearrange(
                            "(kt p) n -> p kt n", p=128
                        ),
                    )
                # mi-outer so pss[0]'s eviction overlaps pss[1]'s matmuls
                for mi in range(2):
                    for kt in range(KT):
                        for hf in range(2):
                            nc.tensor.matmul(
                                pss[mi][:, 512 * hf : 512 * hf + w],
                                wt[:, kt, 128 * mi : 128 * (mi + 1)],
                                xT[:, kt, x0 + w * hf : x0 + w * (hf + 1)],
                                start=(kt == 0),
                                stop=(kt == KT - 1),
                            )
                for mi in range(2):
                    m = 2 * mp + mi
                    g = m % 8
                    psv = pss[mi].rearrange("p (b c) -> p b c", c=512)[:, :, 0:w]
                    bias_col = qb2[:, m : m + 1]
                    dst = (qn_sb if m < 8 else k_sb)[:, g]
                    dstv = dst[:, x0 : x0 + 2 * w].rearrange(
                        "p (b c) -> p b c", c=w
                    )
                    nc.scalar.activation(dstv, psv, AF.Identity, bias=bias_col)
                    # Pool squares the evicted fp16 values
                    sq = sqpool.tile([128, LOCAL], F16, tag="sq")
                    sqv = sq[:, 0 : 2 * w].rearrange("p (b c) -> p b c", c=w)
                    nc.gpsimd.tensor_mul(sqv, dstv, dstv)
                    # scatter per-head sumsq rows into the accumulators
                    if m < 8:
                        for hf in range(2):
                            nc.tensor.matmul(
                                nsq16[:, 512 * hf : 512 * hf + 256],
                                sel16[:, g],
                                sqv[:, hf],
                                start=(g == 0),
                                stop=(g == 7),
                            )
                    else:
                        for hf in range(2):
                            nc.tensor.matmul(
                                nsk16[:, 512 * hf : 512 * hf + 272],
                                sel16[:, g],
                                sqv[:, hf],
                                start=(g == 0),
                                stop=(g == 7),
                            )
            # batched norm chains
            nsqv = nsq16.rearrange("p (b c) -> p b c", c=512)[:, :, 0:256]
            nc.scalar.activation(
                rq16.rearrange("p (b c) -> p b c", c=256), nsqv, AF.Sqrt
            )
            nc.vector.tensor_scalar_add(rq16, rq16, EPS)
            nc.vector.reciprocal(rq16, rq16)
            nskv = nsk16.rearrange("p (b c) -> p b c", c=512)[:, :, 0:272]
            nc.scalar.activation(
                rk.rearrange("p (b c) -> p b c", c=272), nskv, AF.Sqrt
            )
            nc.vector.tensor_scalar_add(rk, rk, EPS)
            nc.vector.reciprocal(rk, rk)

        # transpose rk (16, 544) -> rkT (token, head) tiles; normalize q
        with (
            tc.tile_pool(name="tp", bufs=2, space="PSUM") as tpp,
            tc.tile_pool(name="bcp", bufs=2, space="PSUM") as bcp,
        ):
            for c in range(5):
                w = 128 if c < 4 else LOCAL - 512
                tp = tpp.tile([128, 16], F32, tag="tp")
                nc.tensor.transpose(tp[0:w], rk[:, 128 * c : 128 * c + w], ident)
                nc.vector.tensor_copy(rkT[0:w, c], tp[0:w])
            rqv = rq16.rearrange("p (b c) -> p b c", c=256)
            for g in range(8):
                bc = bcp.tile([128, 1024], F32, tag="bc", name="bc")
                for hf in range(2):
                    nc.tensor.matmul(
                        bc[:, 512 * hf : 512 * hf + 256],
                        selT16[:, g],
                        rqv[:, hf],
                        start=True,
                        stop=True,
                    )
                bcv = bc.rearrange("p (b c) -> p b c", c=512)[:, :, 0:256]
                dstv = qn_sb[:, g][:, 16:528].rearrange("p (b c) -> p b c", c=256)
                nc.vector.tensor_tensor(dstv, dstv, bcv, OP.mult)

        # ---- phase 1c + 2 + 3: v-proj overlapped with attention; ------
        # ---- out-projection interleaved per query block ----------------
        wv = pp.tile([128, KT, D], F16)              # Wv^T (in-feat, v-feat)
        nc.sync.dma_start(wv, T["wv"].rearrange("(kt p) n -> p kt n", p=128))
        nc.sync.dma_start(vbb, T["vbb"])
        nc.sync.dma_start(v_h[:, :, :, 64:65], T["vones"][:, :, :, None])
        nc.sync.dma_start(masks, T["masks"])
        ow = pp.tile([128, KT, D], F16)              # out_w^T (feat, out)
        nc.sync.dma_start(ow, T["ow"].rearrange("(kt p) n -> p kt n", p=128))
        nc.sync.dma_start(ob, T["ob"])

        def v_tile(vps, m):
            M = 128 if m < 4 else LOCAL - 512
            vp = vps.tile([128, 1024], F32, tag="vps", name="vp")
            for nh in range(2):
                for kt in range(KT):
                    nc.tensor.matmul(
                        vp[0:M, 512 * nh : 512 * (nh + 1)],
                        xT[:, kt, 128 * m : 128 * m + M],
                        wv[:, kt, 512 * nh : 512 * (nh + 1)],
                        start=(kt == 0),
                        stop=(kt == KT - 1),
                    )
                vpv = vp[0:M, 512 * nh : 512 * (nh + 1)].rearrange(
                    "p (h c) -> p h c", c=64
                )
                vbv = vbb[0:M, 512 * nh : 512 * (nh + 1)].rearrange(
                    "p (h c) -> p h c", c=64
                )
                nc.vector.tensor_tensor(
                    v_h[0:M, m, 8 * nh : 8 * (nh + 1), 0:64], vpv, vbv, OP.add
                )

        it = 0
        ot2_box = [None]

        def att(stp, otp, dnp, rrp, h, j):
            nonlocal it
            g, a = h // 2, h % 2
            q0 = 256 * j
            kx = k_sb[64 * a : 64 * a + 64]
            qx = qn_sb[64 * a : 64 * a + 64]
            st = stp.tile([128, 512], F32, tag="st", name="st")
            # score matmuls compute only each chunk's live query window:
            # A keys see q [0:160), B keys q [96:256), C keys q [224:256)
            # (C lives at st[0:32, 320:352] of the same PSUM bank)
            nc.tensor.matmul(
                st[:, 0:144],
                kx[:, g, q0 : q0 + 128],
                qx[:, g, 16 + q0 : 16 + q0 + 144],
                start=True, stop=True,
            )
            nc.tensor.matmul(
                st[:, 352:512],
                kx[:, g, q0 + 128 : q0 + 256],
                qx[:, g, 16 + q0 + 96 : 16 + q0 + 256],
                start=True, stop=True,
            )
            nc.tensor.matmul(
                st[0:32, 320:352],
                kx[:, g, q0 + 256 : q0 + 288],
                qx[:, g, 16 + q0 + 224 : 16 + q0 + 256],
                start=True, stop=True,
            )
            pab = pabs[it % 6]
            it += 1
            # exp with per-key scale 1/(|k|+eps), then 0/1 mask multiply
            nc.scalar.activation(
                pab[:, 0:144], st[:, 0:144], AF.Exp,
                scale=rkT[:, 2 * j, h : h + 1],
            )
            nc.scalar.activation(
                pab[:, 352:512], st[:, 352:512], AF.Exp,
                scale=rkT[:, 2 * j + 1, h : h + 1],
            )
            nc.scalar.activation(
                pab[0:32, 320:352], st[0:32, 320:352], AF.Exp,
                scale=rkT[0:32, 2 * j + 2, h : h + 1],
            )
            # 0/1 mask multiplies: all-SBUF fp16; Pool offload in the j=1
            # window where DVE saturates (Pool cannot touch PSUM, this is
            # one of the few all-SBUF ops it can take)
            mm = nc.gpsimd.tensor_mul if j == 1 else (
                lambda o, a_, b_: nc.vector.tensor_tensor(o, a_, b_, OP.mult)
            )
            mm(pab[:, 0:144], pab[:, 0:144], masks[:, j, 0:144])
            mm(pab[:, 352:512], pab[:, 352:512], masks[:, j, 352:512])
            mm(pab[0:32, 320:352], pab[0:32, 320:352], masks[0:32, j, 320:352])
            # AV for heads (2g, 2g+1) packs into one (65, 512) PSUM tile
            if a == 0:
                ot2_box[0] = otp.tile([65, 512], F32, tag="ot", name="ot")
            ot = ot2_box[0][:, 256 * a : 256 * (a + 1)]
            nc.tensor.matmul(
                ot, v_h[:, 2 * j, h], pab[:, 0:256], start=True, stop=False
            )
            nc.tensor.matmul(
                ot[:, 96:256], v_h[:, 2 * j + 1, h], pab[:, 352:512],
                start=False, stop=False,
            )
            nc.tensor.matmul(
                ot[:, 224:256], v_h[0:32, 2 * j + 2, h], pab[0:32, 320:352],
                start=False, stop=True,
            )
            if a == 1:
                ot2 = ot2_box[0]
                rr = rrp.tile([1, 512], FR, tag="rr", name="rr")
                nc.vector.reciprocal(rr, ot2[64:65])
                dn = dnp.tile([64, 512], F32, tag="dn", name="dn")
                nc.tensor.matmul(dn, ones1r, rr, start=True, stop=True)
                otS = rrp.tile([64, 512], F16, tag="otS", name="otS")
                if j == 0:
                    nc.scalar.activation(otS, ot2[0:64], AF.Copy)
                else:
                    nc.vector.tensor_copy(otS, ot2[0:64])
                for aa in range(2):
                    nc.vector.tensor_tensor(
                        outTn[64 * aa : 64 * aa + 64, g, q0 : q0 + 256],
                        otS[:, 256 * aa : 256 * (aa + 1)],
                        dn[:, 256 * aa : 256 * (aa + 1)],
                        OP.mult,
                    )

        def outproj(yps, ysbp, qb):
            yp = yps.tile([128, 1024], F32, tag="yps", name="yp")
            for nh in range(2):
                for kt in range(KT):
                    nc.tensor.matmul(
                        yp[:, 512 * nh : 512 * (nh + 1)],
                        outTn[:, kt, 128 * qb : 128 * (qb + 1)],
                        ow[:, kt, 512 * nh : 512 * (nh + 1)],
                        start=(kt == 0),
                        stop=False,
                    )
                nc.tensor.matmul(
                    yp[:, 512 * nh : 512 * (nh + 1)],
                    ones1,
                    ob[:, 512 * nh : 512 * (nh + 1)],
                    start=False,
                    stop=True,
                )
            ysb = ysbp.tile([128, 1024], F16, tag="ysb", name="ysb")
            nc.scalar.activation(ysb, yp, AF.Copy)
            nc.sync.dma_start(T["y"][128 * qb : 128 * (qb + 1)], ysb)

        with (
            tc.tile_pool(name="rr", bufs=6) as rrp,
            tc.tile_pool(name="ysb", bufs=3) as ysbp,
            tc.tile_pool(name="stp", bufs=3, space="PSUM") as stp,
            tc.tile_pool(name="otp", bufs=2, space="PSUM") as otp,
            tc.tile_pool(name="dnp", bufs=1, space="PSUM") as dnp,
        ):
            with tc.tile_pool(name="vps", bufs=1, space="PSUM") as vps:
                for m in range(3):
                    v_tile(vps, m)
                for h in range(H):
                    if h == 0:
                        v_tile(vps, 3)
                    if h == 2:
                        v_tile(vps, 4)
                    att(stp, otp, dnp, rrp, h, 0)
            with tc.tile_pool(name="yps", bufs=1, space="PSUM") as yps:
                for h in range(H):
                    if h == 2:
                        outproj(yps, ysbp, 0)
                    if h == 8:
                        outproj(yps, ysbp, 1)
                    att(stp, otp, dnp, rrp, h, 1)
                outproj(yps, ysbp, 2)
                outproj(yps, ysbp, 3)


_PROGRAM = None


def _build_program():
    global _PROGRAM
    if _PROGRAM is not None:
        return _PROGRAM
    nc = bacc.Bacc(
        "TRN2",
        target_bir_lowering=False,
        debug=False,
        enable_asserts=False,
        num_devices=NCORES,
    )
    T = _declare_io(nc)
    with tile.TileContext(nc) as tc:
        with nc.allow_low_precision(reason="fp16/fp32r matmul pipeline"):
            _emit(tc, T)
    nc.compile()
    _PROGRAM = nc
    return nc


def _host_masks(c0):
    """0/1 masks[p, j, col] for the core at chunk start c0.

    Column layout per query block j: [0:160) chunk-A live window (queries
    [q0, q0+160)), [160:320) dead, [320:352) chunk-C window (queries
    [q0+224, q0+256), keys rows 0:32), [352:512) chunk-B window (queries
    [q0+96, q0+256)).
    """
    out = np.zeros((2, 128, 512), dtype=np.float32)
    for j in range(2):
        q0 = 256 * j

        def fill(base_key, rows, cols, q_off):
            qtok = c0 + q0 + q_off + np.arange(cols)[None, :]
            ktok = c0 - HALO + q0 + base_key + np.arange(rows)[:, None]
            diff = ktok - qtok
            ok = (
                (np.abs(diff) <= KWIN * DIL)
                & (diff % DIL == 0)
                & (ktok >= 0)
                & (ktok < N)
            )
            return ok.astype(np.float32)

        out[j, :, 0:160] = fill(0, 128, 160, 0)
        out[j, 0:32, 320:352] = fill(256, 32, 32, 224)
        out[j, :, 352:512] = fill(128, 128, 160, 96)
    return np.ascontiguousarray(out.transpose(1, 0, 2)).astype(np.float16)


def _host_inputs(x, qkv_w, qkv_b, out_w, out_b):
    wqk = np.ascontiguousarray(qkv_w[: 2 * D].T.astype(np.float16))    # (D, 2D)
    wv = np.ascontiguousarray(qkv_w[2 * D :].T.astype(np.float16))     # (D, D)
    ow = np.ascontiguousarray(out_w.T.astype(np.float16))              # (D, D)
    qb2 = np.ascontiguousarray(qkv_b[: 2 * D])
    vbb = np.ascontiguousarray(
        np.broadcast_to(qkv_b[2 * D :].astype(np.float16), (128, D))
    )
    ob = np.ascontiguousarray(out_b.reshape(1, D).astype(np.float16))
    selT16 = np.zeros((16, 8, 128), dtype=np.float32)
    for g in range(8):
        selT16[2 * g, g, :64] = 1.0
        selT16[2 * g + 1, g, 64:] = 1.0
    sel16 = np.zeros((8, 128, 16), dtype=np.float16)
    for g in range(8):
        sel16[g, :64, 2 * g] = 1.0
        sel16[g, 64:, 2 * g + 1] = 1.0
    ones1 = np.ones((1, 128), dtype=np.float16)
    ones1r = np.ones((1, 64), dtype=np.float32)
    vones = np.ones((128, 5, 16), dtype=np.float16)
    ident = np.eye(16, dtype=np.float32)

    in_maps = []
    for c in range(NCORES):
        b, i = divmod(c, 4)
        c0 = CHUNK * i
        xT = np.zeros((D, LOCAL), dtype=np.float16)
        lo, hi = max(0, c0 - HALO), min(N, c0 + CHUNK + HALO)
        xT[:, lo - (c0 - HALO) : hi - (c0 - HALO)] = x[b, lo:hi].T.astype(
            np.float16
        )
        in_maps.append(
            {
                "xT": xT,
                "wqk": wqk,
                "wv": wv,
                "ow": ow,
                "masks": _host_masks(c0),
                "qb2": qb2,
                "vbb": vbb,
                "ob": ob,
                "sel16": sel16,
                "selT16": selT16,
                "ones1": ones1,
                "ones1r": ones1r,
                "vones": vones,
                "ident": ident,
            }
        )
    return in_maps


def kernel(x, qkv_w, qkv_b, out_w, out_b):
    x = np.asarray(x, dtype=np.float32)
    qkv_w = np.asarray(qkv_w, dtype=np.float32)
    qkv_b = np.asarray(qkv_b, dtype=np.float32)
    out_w = np.asarray(out_w, dtype=np.float32)
    out_b = np.asarray(out_b, dtype=np.float32)

    nc = _build_program()
    in_maps = _host_inputs(x, qkv_w, qkv_b, out_w, out_b)
    res = bass_utils.run_bass_kernel_spmd(nc, in_maps, core_ids=list(range(NCORES)))

    out = np.empty((B, N, D), dtype=np.float32)
    for c in range(NCORES):
        b, i = divmod(c, 4)
        out[b, CHUNK * i : CHUNK * (i + 1)] = res.results[c]["y"].astype(np.float32)
    return out


# revision 51
# speedup vs baseline: 1.8086x; 1.0500x over previous
"""Dilated MHSA block on 8 Trainium2 NeuronCores.

Sharding: sequence-parallel. Core c (0..7) handles batch b=c//4, query chunk
[512*(c%4), 512*(c%4)+512) with a 16-token halo of keys/values on each side.
Each core computes its full 512x1024 output slice; the host just concatenates.

All heavy matmuls take fp16 inputs (1 cycle/row) and accumulate in fp32 PSUM.

Per-core device pipeline (v2 — engine-balanced rewrite):
  1a. q,k projection qkT = Wqk @ x^T in (feature, token) layout. ACT evicts
      PSUM with per-feature bias (Identity); the Pool engine squares the
      evicted fp16 values; one-hot selector matmuls scatter per-head sumsq
      rows into two PSUM accumulators (nsq16 for q heads, nsk16 for k heads).
      After the sweep: one sqrt+eps+recip chain per side (vs. per-pair),
      PE transposes rk -> rkT (token, head) for the exp scale trick, and
      per-pair broadcast matmuls + DVE multiplies normalize q.
  1b. v projection in natural (token, feat) layout, bias via rank-1 matmul,
      Pool-engine eviction into 65-stride heads with a ones column so the AV
      matmul also emits the softmax denominator.
  2.  Per (head, 256-query block): 3 score matmuls (keys 128/128/32 on
      partitions; the 32-key tail chunk lives at st[0:32, 320:352] of the
      same PSUM bank), 3 ACT exps (scale = 1/(|k|+eps) per key partition),
      3 DVE multiplies by a 0/1 fp16 mask (all-SBUF fp16 -> fast DVE mode),
      3 AV matmuls packing head pairs into one (65,512) PSUM bank. Pair
      epilogue: DVE reciprocal of the denominator row, rank-1 broadcast
      matmul, and DVE multiplies reading both PSUM operands directly.
      pab dead region [160:256) is zeroed once by memset and never rewritten.
  3.  Output projection y = outT^T @ out_w^T + b (bias as rank-1 matmul),
      ACT-evicted to fp16 and DMA'd out (host upcasts); interleaved per query
      block with the attention sweeps.
"""

import os
import sys

for _p in ("/opt/trn_rl_repo", "/root/.axon_site/_ro/trn_rl_repo"):
    if os.path.isdir(_p) and _p not in sys.path:
        sys.path.insert(0, _p)

import numpy as np

import concourse.bass as bass
import concourse.mybir as mybir
import concourse.tile as tile
from concourse import bacc
from concourse import bass_utils

F32 = mybir.dt.float32
FR = mybir.dt.float32r  # fp32 bits, tf32 matmul mode (full rate at N >= 256)
F16 = mybir.dt.float16

B, N, D = 2, 2048, 1024
H, DH = 16, 64
KWIN, DIL = 8, 2
EPS = 1e-6
NCORES = 8
CHUNK = 512          # queries per core
HALO = 16            # KWIN * DIL
LOCAL = CHUNK + 2 * HALO  # 544 tokens (keys/values) per core
KT = D // 128        # 8 contraction tiles


def _declare_io(nc):
    T = {}

    def inp(name, shape, dt=FR):
        T[name] = nc.dram_tensor(name, shape, dt, kind="ExternalInput").ap()

    inp("xT", (D, LOCAL), F16)
    inp("wqk", (D, 2 * D), F16)
    inp("wv", (D, D), F16)
    inp("ow", (D, D), F16)
    inp("masks", (128, 2, 512), F16)
    inp("qb2", (2 * D,), F32)
    inp("vbb", (128, D), F16)
    inp("ob", (1, D), F16)
    inp("sel16", (8, 128, 16), F16)
    inp("selT16", (16, 8, 128))
    inp("ones1", (1, 128), F16)
    inp("ones1r", (1, 64))
    inp("vones", (128, 5, 16), F16)
    inp("ident", (16, 16), F32)
    T["y"] = nc.dram_tensor("y", (CHUNK, D), F16, kind="ExternalOutput").ap()
    return T


def _emit(tc, T):
    nc = tc.nc
    AF = mybir.ActivationFunctionType
    OP = mybir.AluOpType

    with tc.tile_pool(name="persist", bufs=1) as pp:
        xT = pp.tile([128, KT, LOCAL], F16)          # x^T, (in-feat, token)
        wq_prefetch = {}

        qb2 = pp.tile([128, 16], F32)                # qkv bias for q,k
        sel16 = pp.tile([128, 8, 16], F16)            # per-head scatter selectors
        selT16 = pp.tile([16, 8, 128], FR)           # head-pair bcast selectors
        ones1 = pp.tile([1, 128], F16)
        ones1r = pp.tile([1, 64], FR)
        ident = pp.tile([16, 16], F32)

        v_sb = pp.tile([128, 5, H * 65], F16)         # v natural, 65-stride heads
        v_h = v_sb.rearrange("p m (h c) -> p m h c", c=65)

        pabs = [pp.tile([128, 512], F16, name=f"pab{i}") for i in range(6)]
        masks = pp.tile([128, 2, 512], F16)          # (p, qblock, col) 0/1
        vbb = pp.tile([128, D], F16)                 # v bias, host-broadcast
        ob = pp.tile([1, D], F16)

        k_sb = pp.tile([128, 8, LOCAL], F16)          # k^T (biased), 2 heads/tile
        qn_sb = pp.tile([128, 8, LOCAL], F16)         # q^T normalized
        outTn = pp.tile([128, KT, CHUNK], F16)        # attn out^T (feat, q)
        rkT = pp.tile([128, 5, H], F32)              # 1/(|k|+eps), (token, head)
        rk = pp.tile([16, LOCAL], F32)
        rq16 = pp.tile([16, CHUNK], FR)              # 1/(|q|+eps), (head, q)

        # zero the pab dead regions once; never rewritten. [144:256) is read
        # by the A-chunk AV matmul; [320:352) rows 32:128 by the merged B+C
        # mask multiply.
        for i in range(6):
            nc.vector.memset(pabs[i][:, 144:256], 0.0)
            nc.vector.memset(pabs[i][:, 320:352], 0.0)

        # ---- phase 1a: q,k projection + norm reductions -----------------
        with (
            tc.tile_pool(name="wpool", bufs=8) as wpool,
            tc.tile_pool(name="sqpool", bufs=4) as sqpool,
            tc.tile_pool(name="ps1", bufs=2, space="PSUM") as ps1,
            tc.tile_pool(name="nsp", bufs=1, space="PSUM") as nsp,
        ):
            # startup-critical DMA order: first weight k-tile, then x k-tiles
            # interleaved with the rest of the first two weight tiles, so the
            # PE can start at ~1us instead of waiting for full transfers.
            wts = [
                wpool.tile([128, KT, 256], F16, tag="wqk", name=f"wtp{mp}")
                for mp in range(2)
            ]
            wq_prefetch.update(enumerate(wts))
            wqkv = T["wqk"].rearrange("(kt p) n -> p kt n", p=128)
            xTv = T["xT"].rearrange("(kt p) n -> p kt n", p=128)
            for k0, k1 in ((0, 2), (2, 4), (4, 8)):
                nc.sync.dma_start(wts[0][:, k0:k1], wqkv[:, k0:k1, 0:256])
                nc.sync.dma_start(xT[:, k0:k1], xTv[:, k0:k1])
                nc.sync.dma_start(wts[1][:, k0:k1], wqkv[:, k0:k1, 256:512])
            nc.sync.dma_start(qb2, T["qb2"].rearrange("(m p) -> p m", p=128))
            nc.sync.dma_start(sel16, T["sel16"].rearrange("g p c -> p g c"))
            nc.sync.dma_start(selT16, T["selT16"])
            nc.sync.dma_start(ones1, T["ones1"])
            nc.sync.dma_start(ones1r, T["ones1r"])
            nc.sync.dma_start(ident, T["ident"])

            nsq16 = nsp.tile([16, 512], F32, name="nsq16")   # q sumsq (head, q)
            nsk16 = nsp.tile([16, 1024], F32, name="nsk16")  # k sumsq (head, tok)

            for mp in range(8):          # pair of feature tiles
                is_q = mp < 4
                # q features need only the 512 live query columns [16:528);
                # k features need all 544 (keys include the halo)
                w = 256 if is_q else 272
                x0 = 16 if is_q else 0
                pss = [
                    ps1.tile([128, 1024], F32, tag="qkps", name=f"qkps{i}")
                    for i in range(2)
                ]
                if mp in wq_prefetch:
                    wt = wq_prefetch.pop(mp)
                else:
                    wt = wpool.tile([128, KT, 256], F16, tag="wqk")
                    nc.sync.dma_start(
                        wt,
                        T["wqk"][:, 256 * mp : 256 * (mp + 1)].rearrange(
                            "(kt p) n -> p kt n", p=128
                        ),
                    )
                # mi-outer so pss[0]'s eviction overlaps pss[1]'s matmuls;
                # q tiles (512 live queries) fit one PSUM bank -> one matmul
                for mi in range(2):
                    for kt in range(KT):
                        if is_q:
                            nc.tensor.matmul(
                                pss[mi][:, 0:512],
                                wt[:, kt, 128 * mi : 128 * (mi + 1)],
                                xT[:, kt, 16:528],
                                start=(kt == 0),
                                stop=(kt == KT - 1),
                            )
                        else:
                            for hf in range(2):
                                nc.tensor.matmul(
                                    pss[mi][:, 512 * hf : 512 * hf + w],
                                    wt[:, kt, 128 * mi : 128 * (mi + 1)],
                                    xT[:, kt, w * hf : w * (hf + 1)],
                                    start=(kt == 0),
                                    stop=(kt == KT - 1),
                                )
                for mi in range(2):
                    m = 2 * mp + mi
                    g = m % 8
                    bias_col = qb2[:, m : m + 1]
                    sq = sqpool.tile([128, LOCAL], F16, tag="sq")
                    if m < 8:
                        psv = pss[mi][:, 0:512]
                        dstv = qn_sb[:, g][:, 16:528]
                        sqv = sq[:, 0:512]
                        nc.scalar.activation(dstv, psv, AF.Identity, bias=bias_col)
                        nc.gpsimd.tensor_mul(sqv, dstv, dstv)
                        nc.tensor.matmul(
                            nsq16, sel16[:, g], sqv,
                            start=(g == 0), stop=(g == 7),
                        )
                    else:
                        psv = pss[mi].rearrange("p (b c) -> p b c", c=512)[:, :, 0:w]
                        dstv = k_sb[:, g].rearrange("p (b c) -> p b c", c=w)
                        sqv = sq[:, 0 : 2 * w].rearrange("p (b c) -> p b c", c=w)
                        nc.scalar.activation(dstv, psv, AF.Identity, bias=bias_col)
                        nc.gpsimd.tensor_mul(sqv, dstv, dstv)
                        for hf in range(2):
                            nc.tensor.matmul(
                                nsk16[:, 512 * hf : 512 * hf + 272],
                                sel16[:, g],
                                sqv[:, hf],
                                start=(g == 0),
                                stop=(g == 7),
                            )
            # batched norm chains
            nc.scalar.activation(rq16, nsq16, AF.Sqrt)
            nc.vector.tensor_scalar_add(rq16, rq16, EPS)
            nc.vector.reciprocal(rq16, rq16)
            nskv = nsk16.rearrange("p (b c) -> p b c", c=512)[:, :, 0:272]
            nc.scalar.activation(
                rk.rearrange("p (b c) -> p b c", c=272), nskv, AF.Sqrt
            )
            nc.vector.tensor_scalar_add(rk, rk, EPS)
            nc.vector.reciprocal(rk, rk)

        # transpose rk (16, 544) -> rkT (token, head) tiles; normalize q
        with (
            tc.tile_pool(name="tp", bufs=2, space="PSUM") as tpp,
            tc.tile_pool(name="bcp", bufs=2, space="PSUM") as bcp,
        ):
            for c in range(5):
                w = 128 if c < 4 else LOCAL - 512
                tp = tpp.tile([128, 16], F32, tag="tp")
                nc.tensor.transpose(tp[0:w], rk[:, 128 * c : 128 * c + w], ident)
                nc.vector.tensor_copy(rkT[0:w, c], tp[0:w])
            for g in range(8):
                bc = bcp.tile([128, 512], F32, tag="bc", name="bc")
                nc.tensor.matmul(bc, selT16[:, g], rq16, start=True, stop=True)
                dstv = qn_sb[:, g][:, 16:528]
                nc.vector.tensor_tensor(dstv, dstv, bc, OP.mult)

        # ---- phase 1c + 2 + 3: v-proj overlapped with attention; ------
        # ---- out-projection interleaved per query block ----------------
        wv = pp.tile([128, KT, D], F16)              # Wv^T (in-feat, v-feat)
        nc.sync.dma_start(wv, T["wv"].rearrange("(kt p) n -> p kt n", p=128))
        nc.sync.dma_start(vbb, T["vbb"])
        nc.sync.dma_start(v_h[:, :, :, 64:65], T["vones"][:, :, :, None])
        nc.sync.dma_start(masks, T["masks"])
        ow = pp.tile([128, KT, D], F16)              # out_w^T (feat, out)
        nc.sync.dma_start(ow, T["ow"].rearrange("(kt p) n -> p kt n", p=128))
        nc.sync.dma_start(ob, T["ob"])

        def v_tile(vps, m):
            M = 128 if m < 4 else LOCAL - 512
            vp = vps.tile([128, 1024], F32, tag="vps", name="vp")
            # kt outer so both nh matmuls share one Ldweights of the xT tile
            for kt in range(KT):
                for nh in range(2):
                    nc.tensor.matmul(
                        vp[0:M, 512 * nh : 512 * (nh + 1)],
                        xT[:, kt, 128 * m : 128 * m + M],
                        wv[:, kt, 512 * nh : 512 * (nh + 1)],
                        start=(kt == 0),
                        stop=(kt == KT - 1),
                    )
            vpv = vp[0:M].rearrange("p (h c) -> p h c", c=64)
            vbv = vbb[0:M].rearrange("p (h c) -> p h c", c=64)
            nc.vector.tensor_tensor(v_h[0:M, m, :, 0:64], vpv, vbv, OP.add)

        it = 0
        ot2_box = [None]

        def att(stp, otp, dnp, rrp, h, j):
            nonlocal it
            g, a = h // 2, h % 2
            q0 = 256 * j
            kx = k_sb[64 * a : 64 * a + 64]
            qx = qn_sb[64 * a : 64 * a + 64]
            st = stp.tile([128, 512], F32, tag="st", name="st")
            # score matmuls compute only each chunk's live query window:
            # A keys see q [0:160), B keys q [96:256), C keys q [224:256)
            # (C lives at st[0:32, 320:352] of the same PSUM bank)
            nc.tensor.matmul(
                st[:, 0:144],
                kx[:, g, q0 : q0 + 128],
                qx[:, g, 16 + q0 : 16 + q0 + 144],
                start=True, stop=True,
            )
            nc.tensor.matmul(
                st[:, 352:512],
                kx[:, g, q0 + 128 : q0 + 256],
                qx[:, g, 16 + q0 + 96 : 16 + q0 + 256],
                start=True, stop=True,
            )
            nc.tensor.matmul(
                st[0:32, 320:352],
                kx[:, g, q0 + 256 : q0 + 288],
                qx[:, g, 16 + q0 + 224 : 16 + q0 + 256],
                start=True, stop=True,
            )
            pab = pabs[it % 6]
            it += 1
            # exp with per-key scale 1/(|k|+eps), then 0/1 mask multiply
            nc.scalar.activation(
                pab[:, 0:144], st[:, 0:144], AF.Exp,
                scale=rkT[:, 2 * j, h : h + 1],
            )
            nc.scalar.activation(
                pab[:, 352:512], st[:, 352:512], AF.Exp,
                scale=rkT[:, 2 * j + 1, h : h + 1],
            )
            nc.scalar.activation(
                pab[0:32, 320:352], st[0:32, 320:352], AF.Exp,
                scale=rkT[0:32, 2 * j + 2, h : h + 1],
            )
            # 0/1 mask multiplies: all-SBUF fp16; Pool offload in the j=1
            # window where DVE saturates (Pool cannot touch PSUM, this is
            # one of the few all-SBUF ops it can take)
            mm = nc.gpsimd.tensor_mul if j == 1 else (
                lambda o, a_, b_: nc.vector.tensor_tensor(o, a_, b_, OP.mult)
            )
            mm(pab[:, 0:144], pab[:, 0:144], masks[:, j, 0:144])
            mm(pab[:, 320:512], pab[:, 320:512], masks[:, j, 320:512])
            # AV for heads (2g, 2g+1) packs into one (65, 512) PSUM tile
            if a == 0:
                ot2_box[0] = otp.tile([65, 512], F32, tag="ot", name="ot")
            ot = ot2_box[0][:, 256 * a : 256 * (a + 1)]
            nc.tensor.matmul(
                ot, v_h[:, 2 * j, h], pab[:, 0:256], start=True, stop=False
            )
            nc.tensor.matmul(
                ot[:, 96:256], v_h[:, 2 * j + 1, h], pab[:, 352:512],
                start=False, stop=False,
            )
            nc.tensor.matmul(
                ot[:, 224:256], v_h[0:32, 2 * j + 2, h], pab[0:32, 320:352],
                start=False, stop=True,
            )
            if a == 1:
                ot2 = ot2_box[0]
                rr = rrp.tile([1, 512], FR, tag="rr", name="rr")
                nc.vector.reciprocal(rr, ot2[64:65])
                dn = dnp.tile([64, 512], F32, tag="dn", name="dn")
                nc.tensor.matmul(dn, ones1r, rr, start=True, stop=True)
                otS = rrp.tile([64, 512], F16, tag="otS", name="otS")
                if j == 0:
                    nc.scalar.activation(otS, ot2[0:64], AF.Copy)
                else:
                    nc.vector.tensor_copy(otS, ot2[0:64])
                for aa in range(2):
                    nc.vector.tensor_tensor(
                        outTn[64 * aa : 64 * aa + 64, g, q0 : q0 + 256],
                        otS[:, 256 * aa : 256 * (aa + 1)],
                        dn[:, 256 * aa : 256 * (aa + 1)],
                        OP.mult,
                    )

        def outproj(yps, ysbp, qb):
            yp = yps.tile([128, 1024], F32, tag="yps", name="yp")
            # kt outer so both nh matmuls share one Ldweights of the outT tile
            for kt in range(KT):
                for nh in range(2):
                    nc.tensor.matmul(
                        yp[:, 512 * nh : 512 * (nh + 1)],
                        outTn[:, kt, 128 * qb : 128 * (qb + 1)],
                        ow[:, kt, 512 * nh : 512 * (nh + 1)],
                        start=(kt == 0),
                        stop=False,
                    )
            for nh in range(2):
                nc.tensor.matmul(
                    yp[:, 512 * nh : 512 * (nh + 1)],
                    ones1,
                    ob[:, 512 * nh : 512 * (nh + 1)],
                    start=False,
                    stop=True,
                )
            ysb = ysbp.tile([128, 1024], F16, tag="ysb", name="ysb")
            nc.scalar.activation(ysb, yp, AF.Copy)
            nc.sync.dma_start(T["y"][128 * qb : 128 * (qb + 1)], ysb)

        with (
            tc.tile_pool(name="rr", bufs=6) as rrp,
            tc.tile_pool(name="ysb", bufs=3) as ysbp,
            tc.tile_pool(name="stp", bufs=3, space="PSUM") as stp,
            tc.tile_pool(name="otp", bufs=2, space="PSUM") as otp,
            tc.tile_pool(name="dnp", bufs=1, space="PSUM") as dnp,
        ):
            with tc.tile_pool(name="vps", bufs=1, space="PSUM") as vps:
                for m in range(3):
                    v_tile(vps, m)
                for h in range(H):
                    if h == 0:
                        v_tile(vps, 3)
                    if h == 2:
                        v_tile(vps, 4)
                    att(stp, otp, dnp, rrp, h, 0)
            with tc.tile_pool(name="yps", bufs=1, space="PSUM") as yps:
                for h in range(H):
                    if h == 2:
                        outproj(yps, ysbp, 0)
                    if h == 8:
                        outproj(yps, ysbp, 1)
                    att(stp, otp, dnp, rrp, h, 1)
                outproj(yps, ysbp, 2)
                outproj(yps, ysbp, 3)


_PROGRAM = None


def _build_program():
    global _PROGRAM
    if _PROGRAM is not None:
        return _PROGRAM
    nc = bacc.Bacc(
        "TRN2",
        target_bir_lowering=False,
        debug=False,
        enable_asserts=False,
        num_devices=NCORES,
    )
    T = _declare_io(nc)
    with tile.TileContext(nc) as tc:
        with nc.allow_low_precision(reason="fp16/fp32r matmul pipeline"):
            _emit(tc, T)
    nc.compile()
    _PROGRAM = nc
    return nc


def _host_masks(c0):
    """0/1 masks[p, j, col] for the core at chunk start c0.

    Column layout per query block j: [0:160) chunk-A live window (queries
    [q0, q0+160)), [160:320) dead, [320:352) chunk-C window (queries
    [q0+224, q0+256), keys rows 0:32), [352:512) chunk-B window (queries
    [q0+96, q0+256)).
    """
    out = np.zeros((2, 128, 512), dtype=np.float32)
    for j in range(2):
        q0 = 256 * j

        def fill(base_key, rows, cols, q_off):
            qtok = c0 + q0 + q_off + np.arange(cols)[None, :]
            ktok = c0 - HALO + q0 + base_key + np.arange(rows)[:, None]
            diff = ktok - qtok
            ok = (
                (np.abs(diff) <= KWIN * DIL)
                & (diff % DIL == 0)
                & (ktok >= 0)
                & (ktok < N)
            )
            return ok.astype(np.float32)

        out[j, :, 0:160] = fill(0, 128, 160, 0)
        out[j, 0:32, 320:352] = fill(256, 32, 32, 224)
        out[j, :, 352:512] = fill(128, 128, 160, 96)
    return np.ascontiguousarray(out.transpose(1, 0, 2)).astype(np.float16)


def _host_inputs(x, qkv_w, qkv_b, out_w, out_b):
    wqk = np.ascontiguousarray(qkv_w[: 2 * D].T.astype(np.float16))    # (D, 2D)
    wv = np.ascontiguousarray(qkv_w[2 * D :].T.astype(np.float16))     # (D, D)
    ow = np.ascontiguousarray(out_w.T.astype(np.float16))              # (D, D)
    qb2 = np.ascontiguousarray(qkv_b[: 2 * D])
    vbb = np.ascontiguousarray(
        np.broadcast_to(qkv_b[2 * D :].astype(np.float16), (128, D))
    )
    ob = np.ascontiguousarray(out_b.reshape(1, D).astype(np.float16))
    selT16 = np.zeros((16, 8, 128), dtype=np.float32)
    for g in range(8):
        selT16[2 * g, g, :64] = 1.0
        selT16[2 * g + 1, g, 64:] = 1.0
    sel16 = np.zeros((8, 128, 16), dtype=np.float16)
    for g in range(8):
        sel16[g, :64, 2 * g] = 1.0
        sel16[g, 64:, 2 * g + 1] = 1.0
    ones1 = np.ones((1, 128), dtype=np.float16)
    ones1r = np.ones((1, 64), dtype=np.float32)
    vones = np.ones((128, 5, 16), dtype=np.float16)
    ident = np.eye(16, dtype=np.float32)

    in_maps = []
    for c in range(NCORES):
        b, i = divmod(c, 4)
        c0 = CHUNK * i
        xT = np.zeros((D, LOCAL), dtype=np.float16)
        lo, hi = max(0, c0 - HALO), min(N, c0 + CHUNK + HALO)
        xT[:, lo - (c0 - HALO) : hi - (c0 - HALO)] = x[b, lo:hi].T.astype(
            np.float16
        )
        in_maps.append(
            {
                "xT": xT,
                "wqk": wqk,
                "wv": wv,
                "ow": ow,
                "masks": _host_masks(c0),
                "qb2": qb2,
                "vbb": vbb,
                "ob": ob,
                "sel16": sel16,
                "selT16": selT16,
                "ones1": ones1,
                "ones1r": ones1r,
                "vones": vones,
                "ident": ident,
            }
        )
    return in_maps


def kernel(x, qkv_w, qkv_b, out_w, out_b):
    x = np.asarray(x, dtype=np.float32)
    qkv_w = np.asarray(qkv_w, dtype=np.float32)
    qkv_b = np.asarray(qkv_b, dtype=np.float32)
    out_w = np.asarray(out_w, dtype=np.float32)
    out_b = np.asarray(out_b, dtype=np.float32)

    nc = _build_program()
    in_maps = _host_inputs(x, qkv_w, qkv_b, out_w, out_b)
    res = bass_utils.run_bass_kernel_spmd(nc, in_maps, core_ids=list(range(NCORES)))

    out = np.empty((B, N, D), dtype=np.float32)
    for c in range(NCORES):
        b, i = divmod(c, 4)
        out[b, CHUNK * i : CHUNK * (i + 1)] = res.results[c]["y"].astype(np.float32)
    return out
